# revision 29
# baseline (speedup 1.0000x reference)
"""CXLoss kernel for 8 Trainium2 NeuronCores.

Math (per sample n):
  meanT = featureT.mean(axis=(0,2,3))                      (global over batch)
  fT = normalize(featureT[n] - meanT), fI = normalize(featureI[n] - meanT)
  S[q,p] = fI[:,q] . fT[:,p]    (C=256 contraction; p,q in [0,4096))
  raw = (1-S)/2 ; div[q] = min_p raw ; W = exp((1 - raw/(div+eps))/sigma)
  CX = W / (sum_p W + eps) ; out[p] = max_q CX ; loss = mean_n -log(mean_p out + eps)

Sharding: core k handles sample n=k//2 and half of the q axis (h=k%2).
The global channel mean of featureT is computed host-side during input
sharding and passed to every core as a tiny extra input, so the cores are
fully independent (no collective, no cross-core rendezvous).

Per-core device pipeline (all f16 matmul operands, f32 PSUM):
  pre:  sqT = Square(fT+negm); ssq via ones-gram on PE; rt = 1/(|fT_c|+eps)
        fTn = (fT+negm)*rt broadcast (DVE STT, f16); fIc = (fI+negm) f16
        ri folded into per-q stats (nhri, hisri) - zero per-element cost.
  main, per 128-q tile (3-stage software pipeline):
        PE:  S_psum = fIc_tile^T @ fTn  (16 MMs, N=512, f16 -> 8 PSUM banks)
        evac to f16 SBUF: first NDV 512-chunks via DVE tensor_copy, the
             rest via ACT Copy; per-chunk DVE reduce_max (f16 512-wide
             hits the 2x mode) -> smax
        stats: divp = nhri*smax + (0.5+eps); rdiv = 1/divp (DVE);
               scl = hisri*rdiv, gam = IS - (IS/2)*rdiv (ACT)
        ACT: W = exp(scl*S + gam) from the f16 copy, accum_out = Wsum
        DVE: invw = 1/Wsum; cx = W*invw (tensor_scalar, 4x);
             R = max(R, cx) (tensor_tensor, 2x)
  post: R [128,4096] f16 ships to DRAM; host does the 128-way per-p max,
        mean over p, -log, mean over samples (same glue role as the
        baseline's pairwise core combine).
"""

import sys
import os

sys.path.insert(0, "/opt/trn_rl_repo")

import numpy as np
from contextlib import ExitStack

EPS = 1e-8
SIGMA = 0.1
IS = 1.0 / (SIGMA + EPS)  # inverse sigma

N, C, H, W = 4, 256, 64, 64
HW = H * W            # 4096 (p axis; also full q axis)
QH = HW // 2          # 2048 q per core
P128 = 128
C2 = C // P128        # 2 channel chunks
QT = QH // P128       # 16 q tiles
PC = 8                # p chunks (PSUM banks)
PCW = HW // PC        # 512
NCOLS = HW // P128    # 32 output columns
NEG_INF = -3.0e38

_CACHE = {}


def _build_nc():
    from concourse import bacc, mybir, masks
    from concourse import tile as tile_mod

    f32 = mybir.dt.float32
    f16 = mybir.dt.float16
    AF = mybir.ActivationFunctionType
    OP = mybir.AluOpType
    AX = mybir.AxisListType

    nc = bacc.Bacc(
        "TRN2",
        target_bir_lowering=False,
        debug=False,
        num_devices=8,
    )

    fT_d = nc.dram_tensor("ft", [C2, P128, HW], f32, kind="ExternalInput").ap()
    fI_d = nc.dram_tensor("fi", [C2, P128, QH], f32, kind="ExternalInput").ap()
    nm_d = nc.dram_tensor("nm", [C2, P128, 1], f32, kind="ExternalInput").ap()
    out_d = nc.dram_tensor("cxo", [P128, HW], f16, kind="ExternalOutput").ap()

    with tile_mod.TileContext(nc) as tc, ExitStack() as ctx:
        persist = ctx.enter_context(tc.tile_pool(name="persist", bufs=1))

        # ---------- load raw inputs ----------
        fT_raw = [persist.tile([P128, HW], f32, name=f"ftraw{c}", tag=f"ftraw{c}") for c in range(C2)]
        fI_raw = [persist.tile([P128, QH], f32, name=f"firaw{c}", tag=f"firaw{c}") for c in range(C2)]
        negm = [persist.tile([P128, 1], f32, name=f"negm{c}", tag=f"negm{c}") for c in range(C2)]
        for c in range(C2):
            nc.sync.dma_start(out=negm[c][:], in_=nm_d[c])
        for j in range(4):
            sl = slice(j * HW // 4, (j + 1) * HW // 4)
            for c in range(C2):
                eng = nc.sync if (2 * j + c) % 2 == 0 else nc.scalar
                eng.dma_start(out=fT_raw[c][:, sl], in_=fT_d[c][:, sl])
        for j in range(2):
            sl = slice(j * QH // 2, (j + 1) * QH // 2)
            for c in range(C2):
                eng = nc.sync if (2 * j + c) % 2 == 0 else nc.scalar
                eng.dma_start(out=fI_raw[c][:, sl], in_=fI_d[c][:, sl])

        # persistent matmul operands + per-q stats
        fTn = [persist.tile([P128, HW], f16, name=f"ftn{c}", tag=f"ftn{c}") for c in range(C2)]
        fIc = [persist.tile([P128, QH], f16, name=f"fic{c}", tag=f"fic{c}") for c in range(C2)]
        nhri = persist.tile([P128, QT], f32, name="nhri", tag="nhri")   # -0.5 * ri
        hisri = persist.tile([P128, QT], f32, name="hisri", tag="hisri")  # (IS/2) * ri

        ones_col = persist.tile([P128, 1], f16, name="ones_col", tag="ones_col")
        ones_row = persist.tile([1, P128], f16, name="ones_row", tag="ones_row")
        id32 = persist.tile([P128, P128], f32, name="id32", tag="id32")
        id16 = persist.tile([P128, P128], f16, name="id16", tag="id16")
        nc.any.memset(ones_col[:], 1.0)
        nc.any.memset(ones_row[:], 1.0)
        masks.make_identity(nc, id32[:])
        masks.make_identity(nc, id16[:])

        # ---------- preprocessing: center, norms, normalize ----------
        with ExitStack() as pctx:
            pre = pctx.enter_context(tc.tile_pool(name="pre", bufs=1))
            pps = pctx.enter_context(tc.tile_pool(name="pps", bufs=1, space="PSUM"))

            # squared centered values (fp16) for norm computation
            sqT = [pre.tile([P128, HW], f16, name=f"sqt{c}", tag=f"sqt{c}") for c in range(C2)]
            sqI = [pre.tile([P128, QH], f16, name=f"sqi{c}", tag=f"sqi{c}") for c in range(C2)]
            # chunked so each activation starts as soon as its DMA slice
            # lands instead of waiting for the whole tensor
            for c in range(C2):
                for j in range(4):
                    sl = slice(j * HW // 4, (j + 1) * HW // 4)
                    nc.scalar.activation(
                        sqT[c][:, sl], fT_raw[c][:, sl], AF.Square,
                        bias=negm[c][:], scale=1.0,
                    )
                for j in range(2):
                    sl = slice(j * QH // 2, (j + 1) * QH // 2)
                    # centered fI (f16) is a matmul operand
                    nc.scalar.activation(
                        fIc[c][:, sl], fI_raw[c][:, sl], AF.Identity,
                        bias=negm[c][:], scale=1.0,
                    )
                # squares of fI on DVE (f16 2x) to keep ACT light
                nc.vector.tensor_tensor(
                    out=sqI[c][:], in0=fIc[c][:], in1=fIc[c][:], op=OP.mult
                )

            # ssq per column via per-128-chunk gram with a ones vector:
            # cols 0..31 <- fT, cols 32..47 <- fI
            ssq_ps = pps.tile([P128, NCOLS + QT], f32, name="ssq", tag="ssq")
            for t in range(NCOLS):
                for kc in range(C2):
                    nc.tensor.matmul(
                        ssq_ps[:, t : t + 1],
                        lhsT=sqT[kc][:, t * P128 : (t + 1) * P128],
                        rhs=ones_col[:],
                        start=(kc == 0),
                        stop=(kc == C2 - 1),
                    )
            for t in range(QT):
                for kc in range(C2):
                    nc.tensor.matmul(
                        ssq_ps[:, NCOLS + t : NCOLS + t + 1],
                        lhsT=sqI[kc][:, t * P128 : (t + 1) * P128],
                        rhs=ones_col[:],
                        start=(kc == 0),
                        stop=(kc == C2 - 1),
                    )

            # r = 1/(sqrt(ssq)+eps): [:, :32] = rt (per p), [:, 32:] = ri (per q)
            rr1 = pre.tile([P128, NCOLS + QT], f32, name="rr1", tag="rr1")
            rr2 = pre.tile([P128, NCOLS + QT], f32, name="rr2", tag="rr2")
            rr = pre.tile([P128, NCOLS + QT], f32, name="rr", tag="rr")
            nc.scalar.activation(rr1[:], ssq_ps[:], AF.Sqrt)
            nc.vector.tensor_scalar_add(rr2[:], rr1[:], EPS)
            nc.vector.reciprocal(rr[:], rr2[:])
            nc.vector.tensor_scalar_mul(nhri[:], rr[:, NCOLS:], -0.5)
            nc.vector.tensor_scalar_mul(hisri[:], rr[:, NCOLS:], IS / 2.0)

            # transpose rt -> [32, 128] rows so each 128-run of p is one row
            rtT_ps = pps.tile([NCOLS, P128], f32, name="rtt", tag="rtt")
            nc.tensor.transpose(rtT_ps[:], rr[:, :NCOLS], id32[:])
            rtT = pre.tile([NCOLS, P128], f16, name="rtt_sb", tag="rtt_sb")
            nc.scalar.activation(rtT[:], rtT_ps[:], AF.Copy)
            # flatten to a single row on partition 0 (DMA can cross partitions)
            rt_row = pre.tile([1, HW], f16, name="rt_row", tag="rt_row")
            nc.sync.dma_start(
                out=rt_row[:].rearrange("o (t c) -> o t c", c=P128), in_=rtT[:]
            )

            # broadcast rt along partitions and normalize fT:
            # fTn = (fT_raw + negm) * rt_bcast     (f16 matmul operand)
            for pc in range(4):
                rtb = pps.tile([P128, 1024], f32, name="rtb", tag="rtb", bufs=3)
                for tt in range(2):
                    lo = pc * 1024 + tt * PCW
                    nc.tensor.matmul(
                        rtb[:, tt * PCW : (tt + 1) * PCW],
                        lhsT=ones_row[:],
                        rhs=rt_row[:, lo : lo + PCW],
                        start=True,
                        stop=True,
                    )
                sl = slice(pc * 1024, (pc + 1) * 1024)
                for c in range(C2):
                    nc.vector.scalar_tensor_tensor(
                        out=fTn[c][:, sl],
                        in0=fT_raw[c][:, sl],
                        scalar=negm[c][:],
                        in1=rtb[:],
                        op0=OP.add,
                        op1=OP.mult,
                    )

        # ---------- main loop over q tiles (software pipelined) ----------
        # Per tile t: MMs fill four 2-bank PSUM quarters; DVE row-maxes each
        # quarter straight from PSUM; ACT computes exp directly from PSUM
        # (no f16 staging copy), with per-quarter Wsum accumulators. The
        # R max-update runs two tiles behind: DVE scales W by 1/Wsum (4x
        # tensor_scalar) and a gpsimd-issued accumulate-DMA folds the
        # result into R with max - entirely off the DVE/ACT critical path.
        rpool = ctx.enter_context(tc.tile_pool(name="rpool", bufs=2))
        r_prev = rpool.tile([P128, HW], f16, name="R", tag="R")
        nc.any.memset(r_prev[:], 0.0)

        NDV = 2                 # PSUM chunks evacuated by DVE tensor_copy

        with ExitStack() as mctx:
            mm = mctx.enter_context(tc.tile_pool(name="mm", bufs=3))
            st = mctx.enter_context(tc.tile_pool(name="st", bufs=4))
            sps = mctx.enter_context(tc.tile_pool(name="sps", bufs=4, space="PSUM"))

            s16_t = [None] * QT     # f16 staging copy of S
            smax_t = [None] * QT    # row max of S
            rdiv_t = [None] * QT
            wt_t = [None] * QT      # W = exp(...)
            wsum_t = [None] * QT
            invw_t = [None] * QT

            for it in range(QT + 2):
                # divp/rdiv for tile t-1 go first in the ACT/DVE queues:
                # their inputs became ready last iteration, so neither
                # engine FIFO head-of-line blocks on them.
                if 1 <= it <= QT:
                    tp = it - 1
                    divp = st.tile([P128, 1], f32, name="divp", tag="divp")
                    nc.scalar.activation(
                        divp[:], smax_t[tp][:], AF.Copy,
                        bias=0.5 + EPS, scale=nhri[:, tp : tp + 1],
                    )
                    rdiv = st.tile([P128, 1], f32, name="rdiv", tag="rdiv")
                    rdiv_t[tp] = rdiv
                    nc.vector.reciprocal(rdiv[:], divp[:])

                # ---- stage A: matmuls + split evacuation + row max, tile t ----
                if it < QT:
                    t = it
                    s16 = mm.tile([P128, HW], f16, name="s16", tag="s16")
                    s16_t[t] = s16
                    ps_g = []
                    for g in range(4):
                        ps = sps.tile([P128, 2 * PCW], f32, name="ps", tag="ps")
                        ps_g.append(ps)
                    for g in range(4):
                        for kc in range(C2):
                            for j in range(2):
                                lo = j * PCW
                                nc.tensor.matmul(
                                    ps_g[g][:, lo : lo + PCW],
                                    lhsT=fIc[kc][:, t * P128 : (t + 1) * P128],
                                    rhs=fTn[kc][
                                        :, g * 2 * PCW + lo : g * 2 * PCW + lo + PCW
                                    ],
                                    start=(kc == 0),
                                    stop=(kc == C2 - 1),
                                )

                    # evacuate: one 512 chunk on DVE, the rest on ACT in
                    # 1024-wide copies (fewer instructions, less +352-cycle
                    # ACT overhead per op)
                    nc.vector.tensor_copy(s16[:, 0:PCW], ps_g[0][:, 0:PCW])
                    nc.scalar.activation(
                        s16[:, PCW : 2 * PCW], ps_g[0][:, PCW : 2 * PCW], AF.Copy
                    )
                    for g in range(1, 4):
                        sl = slice(g * 2 * PCW, (g + 1) * 2 * PCW)
                        nc.scalar.activation(s16[:, sl], ps_g[g][:], AF.Copy)

                    smax_parts = st.tile([P128, 4], f32, name="smaxp", tag="smaxp")
                    for g in range(4):
                        sl = slice(g * 2 * PCW, (g + 1) * 2 * PCW)
                        nc.vector.reduce_max(
                            out=smax_parts[:, g : g + 1], in_=s16[:, sl],
                            axis=AX.X,
                        )
                    smax = st.tile([P128, 1], f32, name="smax", tag="smax")
                    smax_t[t] = smax
                    nc.vector.reduce_max(out=smax[:], in_=smax_parts[:], axis=AX.X)

                # ---- stage B: scl/gam + exp for tile t-1 ----
                if 1 <= it <= QT:
                    tp = it - 1
                    scl = st.tile([P128, 1], f32, name="scl", tag="scl")
                    gam = st.tile([P128, 1], f32, name="gam", tag="gam")
                    nc.scalar.activation(
                        scl[:], rdiv_t[tp][:], AF.Copy, scale=hisri[:, tp : tp + 1]
                    )
                    nc.scalar.activation(
                        gam[:], rdiv_t[tp][:], AF.Copy, bias=IS, scale=-IS / 2.0
                    )
                    wt = mm.tile([P128, HW], f16, name="wt", tag="wt")
                    wsum = st.tile([P128, 1], f32, name="wsum", tag="wsum")
                    wt_t[tp] = wt
                    wsum_t[tp] = wsum
                    nc.scalar.activation(
                        wt[:], s16_t[tp][:], AF.Exp, bias=gam[:], scale=scl[:],
                        accum_out=wsum[:],
                    )

                # ---- stage C: normalize + R max-update for tile t-2 ----
                if it >= 2:
                    tq = it - 2
                    invw = st.tile([P128, 1], f32, name="invw", tag="invw")
                    # Wsum >= 1 (W at the row-min position is exactly 1), so
                    # the reference's +eps is negligible - skip it.
                    nc.vector.reciprocal(invw[:], wsum_t[tq][:])
                    cx = mm.tile([P128, HW], f16, name="cx", tag="cx")
                    r_new = rpool.tile([P128, HW], f16, name="R", tag="R")
                    if tq < QT - 1:
                        nc.vector.tensor_scalar_mul(cx[:], wt_t[tq][:], invw[:])
                        nc.vector.tensor_tensor(
                            out=r_new[:], in0=cx[:], in1=r_prev[:], op=OP.max
                        )
                    else:
                        # last tile: chunk the update and ship each final R
                        # slice immediately so the out-DMA overlaps the drain
                        for j in range(4):
                            sl = slice(j * HW // 4, (j + 1) * HW // 4)
                            nc.vector.tensor_scalar_mul(
                                cx[:, sl], wt_t[tq][:, sl], invw[:]
                            )
                            nc.vector.tensor_tensor(
                                out=r_new[:, sl], in0=cx[:, sl],
                                in1=r_prev[:, sl], op=OP.max,
                            )
                            eng = nc.sync if j % 2 == 0 else nc.scalar
                            eng.dma_start(out=out_d[:, sl], in_=r_new[:, sl])
                    r_prev = r_new

        # R [128, 4096] f16 already shipped to DRAM inside the last tile's
        # chunked update; the 128-way per-p max + mean + log glue runs on
        # the host.

    nc.compile()
    return nc


def _get_nc():
    if "nc" not in _CACHE:
        _CACHE["nc"] = _build_nc()
    return _CACHE["nc"]


def _make_in_maps(featureT, featureI):
    featureT = np.asarray(featureT, dtype=np.float32)
    featureI = np.asarray(featureI, dtype=np.float32)
    # global channel mean of featureT (host side; keeps cores independent)
    negm = (
        -featureT.mean(axis=(0, 2, 3), dtype=np.float64)
        .astype(np.float32)
        .reshape(C2, P128, 1)
    )
    in_maps = []
    for k in range(8):
        n, h = k // 2, k % 2
        ft = np.ascontiguousarray(featureT[n].reshape(C2, P128, HW))
        fi = np.ascontiguousarray(
            featureI[n].reshape(C, HW)[:, h * QH : (h + 1) * QH].reshape(
                C2, P128, QH
            )
        )
        in_maps.append({"ft": ft, "fi": fi, "nm": negm})
    return in_maps


def run(featureT, featureI, trace=False):
    from concourse.bass_utils import run_bass_kernel_spmd

    nc = _get_nc()
    in_maps = _make_in_maps(featureT, featureI)
    res = run_bass_kernel_spmd(nc, in_maps, list(range(8)), trace=trace)
    outs = [np.asarray(res.results[k]["cxo"], dtype=np.float64) for k in range(8)]
    losses = []
    for n in range(N):
        # per-p max over this sample's 2048 q rows (two cores x 128 rows)
        cx = np.maximum(outs[2 * n], outs[2 * n + 1]).max(axis=0)
        losses.append(-np.log(cx.mean() + EPS))
    loss = np.float32(np.mean(losses))
    return loss, res


def kernel(featureT, featureI):
    loss, _ = run(featureT, featureI, trace=False)
    return loss
